# revision 2
# baseline (speedup 1.0000x reference)
"""Trainium2 Bass kernel for nn_AttnDecoderSRU.

Model (see reference): embedding lookup -> 2 SRU layers (matmul + sigmoid
gates + sequential scan over L + highway) -> dot attention over encoder
outputs -> tanh projection -> [512, 32000] vocab projection with bias.

Sharding (8 cores):
  Launch A: data-parallel over batch (4 of 32 batches per core) for the
            SRU scan + attention.  Everything is kept feature-major
            ("xT" = [D, tokens]) so that
              - SRU weights [D, 3D] are directly the matmul lhsT,
              - gate biases become per-partition scalars fused into the
                ScalarE sigmoid,
              - the scan runs along the free dim via the HW
                tensor_tensor_scan instruction.
  Launch B: tensor-parallel over the vocab dim (4000 cols of out_W per
            core, padded to 4096).  Computed transposed
            (outT = out_W^T @ attn_h) so out_b is per-partition and fuses
            into the PSUM evacuation on ScalarE/VectorE for free.

The host only does layout work (row selection for the embedding, reshapes,
transposes, concatenation); every FLOP of the model runs on device.
"""

import numpy as np

import concourse.bass as bass
import concourse.mybir as mybir
import concourse.tile as tile
from concourse import bacc, bass_utils
from concourse.masks import make_identity

F32 = mybir.dt.float32
AF = mybir.ActivationFunctionType
OP = mybir.AluOpType

N_CORES = 8
L, B, S, D, V, NL = 64, 32, 64, 512, 32000, 2
BC = B // N_CORES          # batches per core
T = BC * L                 # tokens per core (b-major, l-minor)
TT = B * L                 # all tokens
KD = D // 128              # 4 k-tiles over D
VC = V // N_CORES          # 4000 vocab cols per core
VCP = 4096                 # padded to whole 128-tiles
MV = VCP // 128            # 32 m-tiles


def build_sru_attn(reps: int = 1):
    """Launch A: embedding (pre-gathered xT) -> 2 SRU layers -> attention."""
    nc = bacc.Bacc("TRN2", target_bir_lowering=False, debug=False,
                   num_devices=N_CORES)
    xT_d = nc.dram_tensor("xT", [KD, 128, T], F32, kind="ExternalInput")
    mem_d = nc.dram_tensor("mem", [BC, S, D], F32, kind="ExternalInput")
    c0_d = nc.dram_tensor("c0T", [NL, KD, 128, BC], F32, kind="ExternalInput")
    w_d = nc.dram_tensor("sruW", [NL, KD, 128, 3 * D], F32, kind="ExternalInput")
    b_d = nc.dram_tensor("sru_bT", [NL, 128, 8], F32, kind="ExternalInput")
    aw_d = nc.dram_tensor("attnWT", [8, 128, D], F32, kind="ExternalInput")
    ah_d = nc.dram_tensor("ahT", [KD, 128, T], F32, kind="ExternalOutput")
    hid_d = nc.dram_tensor("hidT", [NL, KD, 128, BC], F32, kind="ExternalOutput")

    with tile.TileContext(nc) as tc:
        with (
            tc.tile_pool(name="res", bufs=1) as res,
            tc.tile_pool(name="work", bufs=2) as work,
            tc.tile_pool(name="ups", bufs=2, space="PSUM") as ups,
            tc.tile_pool(name="tps", bufs=2, space="PSUM") as tps,
            tc.tile_pool(name="aps", bufs=1, space="PSUM") as aps,
        ):
            ident = res.tile([128, 128], F32, tag="ident", name="ident")
            make_identity(nc, ident[:])

            # resident weights / constants
            wl = [[res.tile([128, 3 * D], F32, tag=f"w{l}{k}", name=f"w{l}{k}") for k in range(KD)]
                  for l in range(NL)]
            for l in range(NL):
                for k in range(KD):
                    nc.sync.dma_start(wl[l][k][:], w_d[l, k])
            bt = [res.tile([128, 8], F32, tag=f"bt{l}", name=f"bt{l}") for l in range(NL)]
            for l in range(NL):
                nc.sync.dma_start(bt[l][:], b_d[l])
            c0 = [[res.tile([128, BC], F32, tag=f"c0{l}{k}", name=f"c0{l}{k}") for k in range(KD)]
                  for l in range(NL)]
            for l in range(NL):
                for k in range(KD):
                    nc.sync.dma_start(c0[l][k][:], c0_d[l, k])
            awt = [res.tile([128, D], F32, tag=f"aw{k}", name=f"aw{k}") for k in range(8)]
            for k in range(8):
                nc.sync.dma_start(awt[k][:], aw_d[k])
            memb = [res.tile([S, D], F32, tag=f"memb{b}", name=f"memb{b}") for b in range(BC)]
            for b in range(BC):
                nc.sync.dma_start(memb[b][:], mem_d[b])
            xk = [res.tile([128, T], F32, tag=f"x{k}", name=f"x{k}") for k in range(KD)]

            for _ in range(reps):
                for k in range(KD):
                    nc.sync.dma_start(xk[k][:], xT_d[k])

                # ---- SRU layers ----
                for l in range(NL):
                    zt = [work.tile([128, T], F32, tag=f"z{k}", name=f"z{k}") for k in range(KD)]
                    ft = [work.tile([128, T], F32, tag=f"f{k}", name=f"f{k}") for k in range(KD)]
                    rt = [work.tile([128, T], F32, tag=f"r{k}", name=f"r{k}") for k in range(KD)]
                    for m in range(12):
                        ps = ups.tile([128, T], F32, tag="u", name="u")
                        for k in range(KD):
                            nc.tensor.matmul(
                                ps[:], lhsT=wl[l][k][:, m * 128:(m + 1) * 128],
                                rhs=xk[k][:], start=(k == 0), stop=(k == KD - 1))
                        if m < 4:       # z pre-activation: plain copy out
                            nc.scalar.copy(zt[m][:], ps[:])
                        elif m < 8:     # f = sigmoid(fp + bf)
                            nc.scalar.activation(ft[m - 4][:], ps[:], AF.Sigmoid,
                                                 bias=bt[l][:, m - 4:m - 3])
                        else:           # r = sigmoid(rp + br)
                            nc.scalar.activation(rt[m - 8][:], ps[:], AF.Sigmoid,
                                                 bias=bt[l][:, m - 4:m - 3])
                    for k in range(KD):
                        # z'' = (f-1)*z ; then scan c = f*c - z''
                        zpp = work.tile([128, T], F32, tag=f"zp{k}", name=f"zp{k}")
                        nc.vector.scalar_tensor_tensor(
                            zpp[:], in0=ft[k][:], scalar=1.0, in1=zt[k][:],
                            op0=OP.subtract, op1=OP.mult)
                        ct = work.tile([128, T], F32, tag=f"c{k}", name=f"c{k}")
                        for b in range(BC):
                            sl = slice(b * L, (b + 1) * L)
                            nc.vector.tensor_tensor_scan(
                                ct[:, sl], data0=ft[k][:, sl], data1=zpp[:, sl],
                                initial=c0[l][k][:, b:b + 1],
                                op0=OP.mult, op1=OP.subtract)
                        nc.sync.dma_start(
                            hid_d[l, k],
                            ct.rearrange("p (b l) -> p b l", l=L)[:, :, L - 1])
                        # highway: h = r*tanh(c) + (1-r)*x  (into xk in place)
                        th = work.tile([128, T], F32, tag=f"t{k}", name=f"t{k}")
                        nc.scalar.activation(th[:], ct[:], AF.Tanh)
                        nc.vector.tensor_sub(th[:], th[:], xk[k][:])
                        nc.vector.tensor_mul(th[:], rt[k][:], th[:])
                        nc.vector.tensor_add(xk[k][:], th[:], xk[k][:])

                # ---- attention ----
                # memT[k][:, b*S:(b+1)*S] = mem_b^T  (PE transpose, 64x128 blocks)
                memT = [res.tile([128, BC * S], F32, tag=f"mT{k}", name=f"mT{k}") for k in range(KD)]
                for k in range(KD):
                    for b in range(BC):
                        tp = tps.tile([128, S], F32, tag="tp", name="tp")
                        nc.tensor.transpose(tp[:], memb[b][:, k * 128:(k + 1) * 128],
                                            ident[:S, :S])
                        nc.vector.tensor_copy(memT[k][:, b * S:(b + 1) * S], tp[:])
                attn_ps = [aps.tile([128, T], F32, tag=f"ap{m}", name=f"ap{m}") for m in range(KD)]
                for b in range(BC):
                    sl = slice(b * L, (b + 1) * L)
                    ssl = slice(b * S, (b + 1) * S)
                    sc = tps.tile([L, S], F32, tag="tp", name="tp")
                    for k in range(KD):
                        nc.tensor.matmul(sc[:], lhsT=xk[k][:, sl],
                                         rhs=memT[k][:, ssl],
                                         start=(k == 0), stop=(k == KD - 1))
                    mx = work.tile([L, 1], F32, tag="mx", name="mx")
                    nc.vector.reduce_max(mx[:], sc[:], axis=mybir.AxisListType.X)
                    nmx = work.tile([L, 1], F32, tag="nmx", name="nmx")
                    nc.vector.tensor_scalar_mul(nmx[:], mx[:], -1.0)
                    es = work.tile([L, S], F32, tag="es", name="es")
                    nc.scalar.activation(es[:], sc[:], AF.Exp, bias=nmx[:])
                    sm = work.tile([L, 1], F32, tag="sm", name="sm")
                    nc.vector.reduce_sum(sm[:], es[:], axis=mybir.AxisListType.X)
                    rp = work.tile([L, 1], F32, tag="rp", name="rp")
                    nc.vector.reciprocal(rp[:], sm[:])
                    al = work.tile([L, S], F32, tag="al", name="al")
                    nc.vector.tensor_scalar_mul(al[:], es[:], rp[:])
                    tpa = tps.tile([S, L], F32, tag="tp", name="tp")
                    nc.tensor.transpose(tpa[:], al[:], ident[:L, :L])
                    alT = work.tile([S, L], F32, tag="alT", name="alT")
                    nc.vector.tensor_copy(alT[:], tpa[:])
                    ck = []
                    for m in range(KD):
                        cp = tps.tile([128, L], F32, tag="tp", name="tp")
                        nc.tensor.matmul(cp[:], lhsT=memb[b][:, m * 128:(m + 1) * 128],
                                         rhs=alT[:], start=True, stop=True)
                        cks = work.tile([128, L], F32, tag=f"ck{m}", name=f"ck{m}")
                        nc.vector.tensor_copy(cks[:], cp[:])
                        ck.append(cks)
                    # attn_hT += attn_W^T @ [ctx; q]
                    for m in range(KD):
                        for k in range(8):
                            rhs = ck[k][:] if k < KD else xk[k - KD][:, sl]
                            nc.tensor.matmul(
                                attn_ps[m][:, sl],
                                lhsT=awt[k][:, m * 128:(m + 1) * 128], rhs=rhs,
                                start=(k == 0), stop=(k == 7))
                for m in range(KD):
                    ah = work.tile([128, T], F32, tag=f"ah{m}", name=f"ah{m}")
                    nc.scalar.activation(ah[:], attn_ps[m][:], AF.Tanh)
                    nc.sync.dma_start(ah_d[m], ah[:])
    nc.compile()
    return nc


def build_vocab(reps: int = 1):
    """Launch B: outT = out_W^T @ attn_h + out_b (vocab-sharded, transposed)."""
    nc = bacc.Bacc("TRN2", target_bir_lowering=False, debug=False,
                   num_devices=N_CORES)
    ah_d = nc.dram_tensor("ah_full", [KD, 128, TT], F32, kind="ExternalInput")
    wv_d = nc.dram_tensor("wv", [KD, 128, VCP], F32, kind="ExternalInput")
    bv_d = nc.dram_tensor("bv", [128, MV], F32, kind="ExternalInput")
    out_d = nc.dram_tensor("outT", [MV, 128, TT], F32, kind="ExternalOutput")

    NSL = TT // 512  # 4 moving slabs of 512
    with tile.TileContext(nc) as tc:
        with (
            tc.tile_pool(name="res", bufs=1) as res,
            tc.tile_pool(name="out", bufs=3) as outp,
            tc.tile_pool(name="ps", bufs=2, space="PSUM") as psp,
        ):
            ah = [res.tile([128, TT], F32, tag=f"ah{k}", name=f"ah{k}") for k in range(KD)]
            wk = [res.tile([128, VCP], F32, tag=f"wk{k}", name=f"wk{k}") for k in range(KD)]
            bv = res.tile([128, MV], F32, tag="bv", name="bv")
            nc.sync.dma_start(bv[:], bv_d[:, :])
            for k in range(KD):
                nc.sync.dma_start(ah[k][:], ah_d[k])
                nc.sync.dma_start(wk[k][:], wv_d[k])
            for _ in range(reps):
                for m in range(MV):
                    ps = psp.tile([128, TT], F32, tag="ps", name="ps")
                    for k in range(KD):
                        for n in range(NSL):
                            nsl = slice(n * 512, (n + 1) * 512)
                            nc.tensor.matmul(
                                ps[:, nsl], lhsT=wk[k][:, m * 128:(m + 1) * 128],
                                rhs=ah[k][:, nsl],
                                start=(k == 0), stop=(k == KD - 1))
                    o = outp.tile([128, TT], F32, tag="o", name="o")
                    # PSUM evacuation + bias, split across ScalarE and VectorE
                    nc.scalar.activation(o[:, 0:TT // 2], ps[:, 0:TT // 2],
                                         AF.Identity, bias=bv[:, m:m + 1])
                    nc.vector.tensor_scalar_add(o[:, TT // 2:TT], ps[:, TT // 2:TT],
                                                bv[:, m:m + 1])
                    nc.sync.dma_start(out_d[m], o[:])
    nc.compile()
    return nc


_CACHE: dict = {}


def _get(name, builder, reps=1):
    key = (name, reps)
    if key not in _CACHE:
        _CACHE[key] = builder(reps)
    return _CACHE[key]


def _run(nc, in_maps, **kw):
    return bass_utils.run_bass_kernel_spmd(nc, in_maps,
                                           core_ids=list(range(N_CORES)), **kw)


def prep_sru_inputs(rnn_input, last_hidden, encoder_outputs, embed_table,
                    sru_W, sru_b, attn_W):
    """Host-side layout only: embedding row-select + transposes/reshapes."""
    rnn_input = np.asarray(rnn_input)
    x = np.asarray(embed_table)[rnn_input]            # [L, B, D] row gather
    sruW_t = np.ascontiguousarray(
        np.asarray(sru_W, np.float32).reshape(NL, KD, 128, 3 * D))
    srub_t = np.ascontiguousarray(
        np.asarray(sru_b, np.float32).reshape(NL, 8, 128).transpose(0, 2, 1))
    attnW_t = np.ascontiguousarray(
        np.asarray(attn_W, np.float32).reshape(8, 128, D))
    maps = []
    for c in range(N_CORES):
        bs = slice(c * BC, (c + 1) * BC)
        xT = np.ascontiguousarray(
            x[:, bs, :].transpose(2, 1, 0).reshape(KD, 128, T))
        mem = np.ascontiguousarray(
            np.asarray(encoder_outputs, np.float32)[:, bs, :].transpose(1, 0, 2))
        c0 = np.ascontiguousarray(
            np.asarray(last_hidden, np.float32)[:, bs, :]
            .transpose(0, 2, 1).reshape(NL, KD, 128, BC))
        maps.append({"xT": xT, "mem": mem, "c0T": c0, "sruW": sruW_t,
                     "sru_bT": srub_t, "attnWT": attnW_t})
    return maps


def prep_vocab_inputs(ah_full, out_W, out_b):
    out_W = np.asarray(out_W, np.float32)
    out_b = np.asarray(out_b, np.float32)
    maps = []
    for c in range(N_CORES):
        wv = np.zeros((D, VCP), np.float32)
        wv[:, :VC] = out_W[:, c * VC:(c + 1) * VC]
        bv = np.zeros((VCP,), np.float32)
        bv[:VC] = out_b[c * VC:(c + 1) * VC]
        maps.append({
            "ah_full": ah_full,
            "wv": np.ascontiguousarray(wv.reshape(KD, 128, VCP)),
            "bv": np.ascontiguousarray(bv.reshape(MV, 128).T),
        })
    return maps


def kernel(rnn_input, last_hidden, encoder_outputs, embed_table,
           sru_W, sru_b, attn_W, out_W, out_b):
    ncA = _get("sru", build_sru_attn)
    ncB = _get("vocab", build_vocab)

    mapsA = prep_sru_inputs(rnn_input, last_hidden, encoder_outputs,
                            embed_table, sru_W, sru_b, attn_W)
    resA = _run(ncA, mapsA).results

    ah_full = np.concatenate([r["ahT"] for r in resA], axis=2)  # [KD,128,TT]
    hidden = np.concatenate(
        [r["hidT"].transpose(0, 3, 1, 2).reshape(NL, BC, D) for r in resA],
        axis=1)                                                  # [NL, B, D]

    mapsB = prep_vocab_inputs(ah_full, out_W, out_b)
    resB = _run(ncB, mapsB).results

    rows = np.concatenate(
        [r["outT"].reshape(VCP, TT)[:VC] for r in resB], axis=0)  # [V, TT]
    output = np.ascontiguousarray(rows.T).reshape(B, L, V)
    return output, hidden


# revision 3
# speedup vs baseline: 81.4930x; 81.4930x over previous
"""Trainium2 Bass kernel for nn_AttnDecoderSRU.

Model (see reference): embedding lookup -> 2 SRU layers (matmul + sigmoid
gates + sequential scan over L + highway) -> dot attention over encoder
outputs -> tanh projection -> [512, 32000] vocab projection with bias.

Sharding (8 cores):
  Launch A: data-parallel over batch (4 of 32 batches per core) for the
            SRU scan + attention.  Everything is kept feature-major
            ("xT" = [D, tokens]) so that
              - SRU weights [D, 3D] are directly the matmul lhsT,
              - gate biases become per-partition scalars fused into the
                ScalarE sigmoid,
              - the scan runs along the free dim via the HW
                tensor_tensor_scan instruction.
  Launch B: tensor-parallel over the vocab dim (4000 cols of out_W per
            core, padded to 4096).  Computed transposed
            (outT = out_W^T @ attn_h) so out_b is per-partition and fuses
            into the PSUM evacuation on ScalarE/VectorE for free.

The host only does layout work (row selection for the embedding, reshapes,
transposes, concatenation); every FLOP of the model runs on device.
"""

import numpy as np

import concourse.bass as bass
import concourse.mybir as mybir
import concourse.tile as tile
from concourse import bacc, bass_utils
from concourse.masks import make_identity

F32 = mybir.dt.float32
AF = mybir.ActivationFunctionType
OP = mybir.AluOpType

N_CORES = 8
L, B, S, D, V, NL = 64, 32, 64, 512, 32000, 2
BC = B // N_CORES          # batches per core
T = BC * L                 # tokens per core (b-major, l-minor)
TT = B * L                 # all tokens
KD = D // 128              # 4 k-tiles over D
VC = V // N_CORES          # 4000 vocab cols per core
VCP = 4096                 # padded to whole 128-tiles
MV = VCP // 128            # 32 m-tiles


def build_sru_attn(reps: int = 1):
    """Launch A: embedding (pre-gathered xT) -> 2 SRU layers -> attention."""
    nc = bacc.Bacc("TRN2", target_bir_lowering=False, debug=False,
                   num_devices=N_CORES)
    xT_d = nc.dram_tensor("xT", [KD, 128, T], F32, kind="ExternalInput")
    mem_d = nc.dram_tensor("mem", [BC, S, D], F32, kind="ExternalInput")
    c0_d = nc.dram_tensor("c0T", [NL, KD, 128, BC], F32, kind="ExternalInput")
    w_d = nc.dram_tensor("sruW", [NL, KD, 128, 3 * D], F32, kind="ExternalInput")
    b_d = nc.dram_tensor("sru_bT", [NL, 128, 8], F32, kind="ExternalInput")
    aw_d = nc.dram_tensor("attnWT", [8, 128, D], F32, kind="ExternalInput")
    ah_d = nc.dram_tensor("ahT", [KD, 128, T], F32, kind="ExternalOutput")
    hid_d = nc.dram_tensor("hidT", [NL, KD, 128, BC], F32, kind="ExternalOutput")

    with tile.TileContext(nc) as tc:
        with (
            tc.tile_pool(name="res", bufs=1) as res,
            tc.tile_pool(name="work", bufs=2) as work,
            tc.tile_pool(name="ups", bufs=2, space="PSUM") as ups,
            tc.tile_pool(name="tps", bufs=2, space="PSUM") as tps,
            tc.tile_pool(name="aps", bufs=1, space="PSUM") as aps,
        ):
            ident = res.tile([128, 128], F32, tag="ident", name="ident")
            make_identity(nc, ident[:])

            # resident weights / constants
            wl = [[res.tile([128, 3 * D], F32, tag=f"w{l}{k}", name=f"w{l}{k}") for k in range(KD)]
                  for l in range(NL)]
            bt = [res.tile([128, 8], F32, tag=f"bt{l}", name=f"bt{l}") for l in range(NL)]
            c0 = [[res.tile([128, BC], F32, tag=f"c0{l}{k}", name=f"c0{l}{k}") for k in range(KD)]
                  for l in range(NL)]
            awt = [res.tile([128, D], F32, tag=f"aw{k}", name=f"aw{k}") for k in range(8)]
            memb = [res.tile([S, D], F32, tag=f"memb{b}", name=f"memb{b}") for b in range(BC)]
            xk = [res.tile([128, T], F32, tag=f"x{k}", name=f"x{k}") for k in range(KD)]

            for rep in range(reps):
                for l in range(NL):
                    for k in range(KD):
                        nc.sync.dma_start(wl[l][k][:], w_d[l, k])
                    nc.sync.dma_start(bt[l][:], b_d[l])
                    for k in range(KD):
                        nc.sync.dma_start(c0[l][k][:], c0_d[l, k])
                for k in range(8):
                    nc.sync.dma_start(awt[k][:], aw_d[k])
                for b in range(BC):
                    nc.sync.dma_start(memb[b][:], mem_d[b])
                for k in range(KD):
                    nc.sync.dma_start(xk[k][:], xT_d[k])

                # ---- SRU layers ----
                for l in range(NL):
                    zt = [work.tile([128, T], F32, tag=f"z{k}", name=f"z{k}") for k in range(KD)]
                    ft = [work.tile([128, T], F32, tag=f"f{k}", name=f"f{k}") for k in range(KD)]
                    rt = [work.tile([128, T], F32, tag=f"r{k}", name=f"r{k}") for k in range(KD)]
                    for m in range(12):
                        ps = ups.tile([128, T], F32, tag="u", name="u")
                        for k in range(KD):
                            nc.tensor.matmul(
                                ps[:], lhsT=wl[l][k][:, m * 128:(m + 1) * 128],
                                rhs=xk[k][:], start=(k == 0), stop=(k == KD - 1))
                        if m < 4:       # z pre-activation: plain copy out
                            nc.scalar.copy(zt[m][:], ps[:])
                        elif m < 8:     # f = sigmoid(fp + bf)
                            nc.scalar.activation(ft[m - 4][:], ps[:], AF.Sigmoid,
                                                 bias=bt[l][:, m - 4:m - 3])
                        else:           # r = sigmoid(rp + br)
                            nc.scalar.activation(rt[m - 8][:], ps[:], AF.Sigmoid,
                                                 bias=bt[l][:, m - 4:m - 3])
                    for k in range(KD):
                        # z'' = (f-1)*z ; then scan c = f*c - z''
                        zpp = work.tile([128, T], F32, tag=f"zp{k}", name=f"zp{k}")
                        nc.vector.scalar_tensor_tensor(
                            zpp[:], in0=ft[k][:], scalar=1.0, in1=zt[k][:],
                            op0=OP.subtract, op1=OP.mult)
                        ct = work.tile([128, T], F32, tag=f"c{k}", name=f"c{k}")
                        for b in range(BC):
                            sl = slice(b * L, (b + 1) * L)
                            nc.vector.tensor_tensor_scan(
                                ct[:, sl], data0=ft[k][:, sl], data1=zpp[:, sl],
                                initial=c0[l][k][:, b:b + 1],
                                op0=OP.mult, op1=OP.subtract)
                        nc.sync.dma_start(
                            hid_d[l, k],
                            ct.rearrange("p (b l) -> p b l", l=L)[:, :, L - 1])
                        # highway: h = r*tanh(c) + (1-r)*x  (into xk in place)
                        th = work.tile([128, T], F32, tag=f"t{k}", name=f"t{k}")
                        nc.scalar.activation(th[:], ct[:], AF.Tanh)
                        nc.vector.tensor_sub(th[:], th[:], xk[k][:])
                        nc.vector.tensor_mul(th[:], rt[k][:], th[:])
                        nc.vector.tensor_add(xk[k][:], th[:], xk[k][:])

                # ---- attention ----
                # memT[k][:, b*S:(b+1)*S] = mem_b^T  (PE transpose, 64x128 blocks)
                memT = [res.tile([128, BC * S], F32, tag=f"mT{k}", name=f"mT{k}") for k in range(KD)]
                for k in range(KD):
                    for b in range(BC):
                        tp = tps.tile([128, S], F32, tag="tp", name="tp")
                        nc.tensor.transpose(tp[:], memb[b][:, k * 128:(k + 1) * 128],
                                            ident[:S, :S])
                        nc.vector.tensor_copy(memT[k][:, b * S:(b + 1) * S], tp[:])
                attn_ps = [aps.tile([128, T], F32, tag=f"ap{m}", name=f"ap{m}") for m in range(KD)]
                for b in range(BC):
                    sl = slice(b * L, (b + 1) * L)
                    ssl = slice(b * S, (b + 1) * S)
                    sc = tps.tile([L, S], F32, tag="tp", name="tp")
                    for k in range(KD):
                        nc.tensor.matmul(sc[:], lhsT=xk[k][:, sl],
                                         rhs=memT[k][:, ssl],
                                         start=(k == 0), stop=(k == KD - 1))
                    mx = work.tile([L, 1], F32, tag="mx", name="mx")
                    nc.vector.reduce_max(mx[:], sc[:], axis=mybir.AxisListType.X)
                    nmx = work.tile([L, 1], F32, tag="nmx", name="nmx")
                    nc.vector.tensor_scalar_mul(nmx[:], mx[:], -1.0)
                    es = work.tile([L, S], F32, tag="es", name="es")
                    nc.scalar.activation(es[:], sc[:], AF.Exp, bias=nmx[:])
                    sm = work.tile([L, 1], F32, tag="sm", name="sm")
                    nc.vector.reduce_sum(sm[:], es[:], axis=mybir.AxisListType.X)
                    rp = work.tile([L, 1], F32, tag="rp", name="rp")
                    nc.vector.reciprocal(rp[:], sm[:])
                    al = work.tile([L, S], F32, tag="al", name="al")
                    nc.vector.tensor_scalar_mul(al[:], es[:], rp[:])
                    tpa = tps.tile([S, L], F32, tag="tp", name="tp")
                    nc.tensor.transpose(tpa[:], al[:], ident[:L, :L])
                    alT = work.tile([S, L], F32, tag="alT", name="alT")
                    nc.vector.tensor_copy(alT[:], tpa[:])
                    ck = []
                    for m in range(KD):
                        cp = tps.tile([128, L], F32, tag="tp", name="tp")
                        nc.tensor.matmul(cp[:], lhsT=memb[b][:, m * 128:(m + 1) * 128],
                                         rhs=alT[:], start=True, stop=True)
                        cks = work.tile([128, L], F32, tag=f"ck{m}", name=f"ck{m}")
                        nc.vector.tensor_copy(cks[:], cp[:])
                        ck.append(cks)
                    # attn_hT += attn_W^T @ [ctx; q]
                    for m in range(KD):
                        for k in range(8):
                            rhs = ck[k][:] if k < KD else xk[k - KD][:, sl]
                            nc.tensor.matmul(
                                attn_ps[m][:, sl],
                                lhsT=awt[k][:, m * 128:(m + 1) * 128], rhs=rhs,
                                start=(k == 0), stop=(k == 7))
                for m in range(KD):
                    ah = work.tile([128, T], F32, tag=f"ah{m}", name=f"ah{m}")
                    nc.scalar.activation(ah[:], attn_ps[m][:], AF.Tanh)
                    nc.sync.dma_start(ah_d[m], ah[:])
    nc.compile()
    return nc


def build_vocab(reps: int = 1):
    """Launch B: outT = out_W^T @ attn_h + out_b (vocab-sharded, transposed)."""
    nc = bacc.Bacc("TRN2", target_bir_lowering=False, debug=False,
                   num_devices=N_CORES)
    ah_d = nc.dram_tensor("ah_full", [KD, 128, TT], F32, kind="ExternalInput")
    wv_d = nc.dram_tensor("wv", [KD, 128, VCP], F32, kind="ExternalInput")
    bv_d = nc.dram_tensor("bv", [128, MV], F32, kind="ExternalInput")
    out_d = nc.dram_tensor("outT", [MV, 128, TT], F32, kind="ExternalOutput")

    NSL = TT // 512  # 4 moving slabs of 512
    with tile.TileContext(nc) as tc:
        with (
            tc.tile_pool(name="res", bufs=1) as res,
            tc.tile_pool(name="out", bufs=3) as outp,
            tc.tile_pool(name="ps", bufs=2, space="PSUM") as psp,
        ):
            ah = [res.tile([128, TT], F32, tag=f"ah{k}", name=f"ah{k}") for k in range(KD)]
            wk = [res.tile([128, VCP], F32, tag=f"wk{k}", name=f"wk{k}") for k in range(KD)]
            bv = res.tile([128, MV], F32, tag="bv", name="bv")
            for _ in range(reps):
                nc.sync.dma_start(bv[:], bv_d[:, :])
                for k in range(KD):
                    nc.sync.dma_start(ah[k][:], ah_d[k])
                    nc.sync.dma_start(wk[k][:], wv_d[k])
                for m in range(MV):
                    ps = psp.tile([128, TT], F32, tag="ps", name="ps")
                    for k in range(KD):
                        for n in range(NSL):
                            nsl = slice(n * 512, (n + 1) * 512)
                            nc.tensor.matmul(
                                ps[:, nsl], lhsT=wk[k][:, m * 128:(m + 1) * 128],
                                rhs=ah[k][:, nsl],
                                start=(k == 0), stop=(k == KD - 1))
                    o = outp.tile([128, TT], F32, tag="o", name="o")
                    # PSUM evacuation + bias, split across ScalarE and VectorE
                    nc.scalar.activation(o[:, 0:TT // 2], ps[:, 0:TT // 2],
                                         AF.Identity, bias=bv[:, m:m + 1])
                    nc.vector.tensor_scalar_add(o[:, TT // 2:TT], ps[:, TT // 2:TT],
                                                bv[:, m:m + 1])
                    nc.sync.dma_start(out_d[m], o[:])
    nc.compile()
    return nc


_CACHE: dict = {}


def _get(name, builder, reps=1):
    key = (name, reps)
    if key not in _CACHE:
        _CACHE[key] = builder(reps)
    return _CACHE[key]


def _run(nc, in_maps, **kw):
    return bass_utils.run_bass_kernel_spmd(nc, in_maps,
                                           core_ids=list(range(N_CORES)), **kw)


def prep_sru_inputs(rnn_input, last_hidden, encoder_outputs, embed_table,
                    sru_W, sru_b, attn_W):
    """Host-side layout only: embedding row-select + transposes/reshapes."""
    rnn_input = np.asarray(rnn_input)
    x = np.asarray(embed_table)[rnn_input]            # [L, B, D] row gather
    sruW_t = np.ascontiguousarray(
        np.asarray(sru_W, np.float32).reshape(NL, KD, 128, 3 * D))
    srub_t = np.ascontiguousarray(
        np.asarray(sru_b, np.float32).reshape(NL, 8, 128).transpose(0, 2, 1))
    attnW_t = np.ascontiguousarray(
        np.asarray(attn_W, np.float32).reshape(8, 128, D))
    maps = []
    for c in range(N_CORES):
        bs = slice(c * BC, (c + 1) * BC)
        xT = np.ascontiguousarray(
            x[:, bs, :].transpose(2, 1, 0).reshape(KD, 128, T))
        mem = np.ascontiguousarray(
            np.asarray(encoder_outputs, np.float32)[:, bs, :].transpose(1, 0, 2))
        c0 = np.ascontiguousarray(
            np.asarray(last_hidden, np.float32)[:, bs, :]
            .transpose(0, 2, 1).reshape(NL, KD, 128, BC))
        maps.append({"xT": xT, "mem": mem, "c0T": c0, "sruW": sruW_t,
                     "sru_bT": srub_t, "attnWT": attnW_t})
    return maps


def prep_vocab_inputs(ah_full, out_W, out_b):
    out_W = np.asarray(out_W, np.float32)
    out_b = np.asarray(out_b, np.float32)
    maps = []
    for c in range(N_CORES):
        wv = np.zeros((D, VCP), np.float32)
        wv[:, :VC] = out_W[:, c * VC:(c + 1) * VC]
        bv = np.zeros((VCP,), np.float32)
        bv[:VC] = out_b[c * VC:(c + 1) * VC]
        maps.append({
            "ah_full": ah_full,
            "wv": np.ascontiguousarray(wv.reshape(KD, 128, VCP)),
            "bv": np.ascontiguousarray(bv.reshape(MV, 128).T),
        })
    return maps


def kernel(rnn_input, last_hidden, encoder_outputs, embed_table,
           sru_W, sru_b, attn_W, out_W, out_b):
    ncA = _get("sru", build_sru_attn)
    ncB = _get("vocab", build_vocab)

    mapsA = prep_sru_inputs(rnn_input, last_hidden, encoder_outputs,
                            embed_table, sru_W, sru_b, attn_W)
    resA = _run(ncA, mapsA).results

    ah_full = np.concatenate([r["ahT"] for r in resA], axis=2)  # [KD,128,TT]
    hidden = np.concatenate(
        [r["hidT"].transpose(0, 3, 1, 2).reshape(NL, BC, D) for r in resA],
        axis=1)                                                  # [NL, B, D]

    mapsB = prep_vocab_inputs(ah_full, out_W, out_b)
    resB = _run(ncB, mapsB).results

    rows = np.concatenate(
        [r["outT"].reshape(VCP, TT)[:VC] for r in resB], axis=0)  # [V, TT]
    output = np.ascontiguousarray(rows.T).reshape(B, L, V)
    return output, hidden
